# revision 1
# baseline (speedup 1.0000x reference)
"""Softmax-weighted nearest-neighbor aggregation (DiffusionStar) on 8 TRN2 cores.

Strategy:
  - Shard the train set (N=50000) across 8 cores (6250 rows each, padded to 6272).
  - Per core, two-phase softmax (both phases DMA-bound, ~38 MB each):
      Phase 1: scores s[b,n] = (2/a_b)<x_b, t_n> - ||t_n||^2 via fp16 GEMM
               (contraction over d, streaming a host-pretransposed fp16 train
               slice); the ||t||^2 row is subtracted in fp32 on DVE and the
               per-group row-max tracked.
      Phase 2: p = exp(g'*(s - M)) on ACT (fp16 out, g' = a^2/(2(1-acp)),
               per-partition scale/bias, fused running sum), p transposed on
               PE, then ACC = p @ train (fp16 GEMM streaming the natural-
               layout slice, fp32 PSUM accumulate across all 49 n-chunks).
               Phase-2 DMA prefetch overlaps the phase boundary.
  - Host merges (M, S, ACC) across cores with the standard online-softmax
    combine and applies the final coefficients in fp64.

Numerics: train/x are cast to fp16 for the GEMMs; products accumulate in fp32
PSUM; ||t||^2 is applied in fp32. Validated end-to-end error ~5e-4 (abs-max
relative) vs the fp32 reference; softmax argmax is preserved.
Measured: ~247-272 us HW exec time on 8 axon-tunneled TRN2 cores.
"""

import numpy as np

B = 64
D = 3072
N = 50000
NCORES = 8
N_LOC = N // NCORES          # 6250
N_PAD = 6272                 # 49 * 128
KD = D // 128                # 24
KN = N_PAD // 128            # 49
DJ = D // 512                # 6
GROUPS = [(i * 512, 512) for i in range(12)] + [(6144, 128)]
PAD_TRSQ = 1e9
NAT_BUFS = 8
INTERLEAVE_P2 = True   # interleave transpose/DMA/GEMM2 per chunk
FUSED_TTR = False      # tensor_tensor_reduce reading PSUM crashes TRN2 HW

_CACHED = {}


def _build_nc():
    import concourse.bacc as bacc
    import concourse.tile as tile
    from concourse import mybir
    from contextlib import ExitStack

    f16 = mybir.dt.float16
    f32 = mybir.dt.float32

    nc = bacc.Bacc("TRN2", target_bir_lowering=False, debug=False)

    tT = nc.dram_tensor("tT", [D, N_PAD], f16, kind="ExternalInput").ap()
    tn = nc.dram_tensor("tn", [N_PAD, D], f16, kind="ExternalInput").ap()
    xT = nc.dram_tensor("xT", [D, B], f16, kind="ExternalInput").ap()
    ident = nc.dram_tensor("ident", [B, B], f16, kind="ExternalInput").ap()
    trsq = nc.dram_tensor("trsq", [B, N_PAD], f32, kind="ExternalInput").ap()
    gcol = nc.dram_tensor("gcol", [B, 1], f32, kind="ExternalInput").ap()

    acc_out = nc.dram_tensor("acc_out", [B, D], f32, kind="ExternalOutput").ap()
    s_out = nc.dram_tensor("s_out", [B, 1], f32, kind="ExternalOutput").ap()
    m_out = nc.dram_tensor("m_out", [B, 1], f32, kind="ExternalOutput").ap()

    NG = len(GROUPS)

    with tile.TileContext(nc) as tc, ExitStack() as ctx:
        const = ctx.enter_context(tc.tile_pool(name="const", bufs=1))
        kTp = ctx.enter_context(tc.tile_pool(name="kT", bufs=2))
        natp = ctx.enter_context(tc.tile_pool(name="nat", bufs=NAT_BUFS))
        sb = ctx.enter_context(tc.tile_pool(name="sb", bufs=1))

        # --- constants ---
        xT_sb = const.tile([128, KD, B], f16)
        nc.sync.dma_start(xT_sb[:], xT.rearrange("(k p) b -> p k b", p=128))
        id_sb = const.tile([B, B], f16)
        nc.sync.dma_start(id_sb[:], ident[:])
        trsq_sb = const.tile([B, N_PAD], f32)
        nc.sync.dma_start(trsq_sb[:], trsq[:])
        g_sb = const.tile([B, 1], f32)
        nc.sync.dma_start(g_sb[:], gcol[:])

        mpart = sb.tile([B, NG], f32)
        ssum = sb.tile([B, NG], f32)
        stat = sb.tile([B, 4], f32)
        acc_sb = sb.tile([B, D], f32)
        sc_tiles = []
        p_tiles = []

        tTr = tT.rearrange("(k p) n -> p k n", p=128)  # [128, KD, N_PAD]

        # --- phase 2 DMA prefetch happens naturally via pool bufs; issue
        #     phase-1 pipeline first ---
        with tc.tile_pool(name="psS", bufs=2, space="PSUM") as psS:
            for gi, (n0, W) in enumerate(GROUPS):
                kT = kTp.tile([128, KD, 512], f16, tag="kT")
                nc.sync.dma_start(kT[:, :, :W], tTr[:, :, n0:n0 + W])
                ps = psS.tile([B, 512], f32, tag="ps")
                for k in range(KD):
                    nc.tensor.matmul(ps[:, :W], xT_sb[:, k, :], kT[:, k, :W],
                                     start=(k == 0), stop=(k == KD - 1))
                sc = sb.tile([B, 512], f32, tag=f"sc{gi}")
                sc_tiles.append(sc)
                if FUSED_TTR:
                    # sc = ps - trsq ; mpart[gi] = rowmax(sc)
                    nc.vector.tensor_tensor_reduce(
                        out=sc[:, :W], in0=ps[:, :W],
                        in1=trsq_sb[:, n0:n0 + W], scale=1.0, scalar=-1e38,
                        op0=mybir.AluOpType.subtract, op1=mybir.AluOpType.max,
                        accum_out=mpart[:, gi:gi + 1])
                else:
                    nc.vector.tensor_tensor(sc[:, :W], ps[:, :W],
                                            trsq_sb[:, n0:n0 + W],
                                            op=mybir.AluOpType.subtract)
                    nc.vector.reduce_max(mpart[:, gi:gi + 1], sc[:, :W],
                                         axis=mybir.AxisListType.X)

        # --- global max, bias = -g*M ---
        nc.vector.reduce_max(stat[:, 0:1], mpart[:, :NG],
                             axis=mybir.AxisListType.X)
        nc.vector.tensor_tensor(stat[:, 2:3], g_sb[:], stat[:, 0:1],
                                op=mybir.AluOpType.mult)
        nc.vector.tensor_scalar_mul(stat[:, 2:3], stat[:, 2:3], -1.0)

        # --- exp -> transpose -> GEMM2, pipelined per group ---
        with tc.tile_pool(name="psT", bufs=2, space="PSUM") as psT, \
             tc.tile_pool(name="psA", bufs=1, space="PSUM") as psA:
            acc_ps = psA.tile([B, DJ, 512], f32)
            pT_tiles = []
            if INTERLEAVE_P2:
                for gi, (n0, W) in enumerate(GROUPS):
                    p = sb.tile([B, 512], f16, tag=f"p{gi}")
                    p_tiles.append(p)
                    nc.scalar.activation(p[:, :W], sc_tiles[gi][:, :W],
                                         mybir.ActivationFunctionType.Exp,
                                         bias=stat[:, 2:3], scale=g_sb[:],
                                         accum_out=ssum[:, gi:gi + 1])
                    for ci in range(W // 128):
                        c = n0 // 128 + ci
                        pt_ps = psT.tile([128, B], f16, tag="pt")
                        nc.tensor.transpose(pt_ps[:],
                                            p[:, ci * 128:(ci + 1) * 128],
                                            id_sb[:])
                        pT = sb.tile([128, B], f16, tag=f"pT{c}")
                        nc.vector.tensor_copy(pT[:], pt_ps[:])
                        nat = natp.tile([128, D], f16, tag="nat")
                        nc.sync.dma_start(nat[:], tn[c * 128:(c + 1) * 128, :])
                        for j in range(DJ):
                            nc.tensor.matmul(acc_ps[:, j, :], pT[:],
                                             nat[:, j * 512:(j + 1) * 512],
                                             start=(c == 0), stop=(c == KN - 1))
            else:
                for gi, (n0, W) in enumerate(GROUPS):
                    p = sb.tile([B, 512], f16, tag=f"p{gi}")
                    p_tiles.append(p)
                    nc.scalar.activation(p[:, :W], sc_tiles[gi][:, :W],
                                         mybir.ActivationFunctionType.Exp,
                                         bias=stat[:, 2:3], scale=g_sb[:],
                                         accum_out=ssum[:, gi:gi + 1])
                for c in range(KN):
                    gi = c // 4
                    ci = c % 4
                    pt_ps = psT.tile([128, B], f16, tag="pt")
                    nc.tensor.transpose(pt_ps[:],
                                        p_tiles[gi][:, ci * 128:(ci + 1) * 128],
                                        id_sb[:])
                    pT = sb.tile([128, B], f16, tag=f"pT{c}")
                    nc.vector.tensor_copy(pT[:], pt_ps[:])
                    pT_tiles.append(pT)
                for c in range(KN):
                    nat = natp.tile([128, D], f16, tag="nat")
                    nc.sync.dma_start(nat[:], tn[c * 128:(c + 1) * 128, :])
                    for j in range(DJ):
                        nc.tensor.matmul(acc_ps[:, j, :], pT_tiles[c][:],
                                         nat[:, j * 512:(j + 1) * 512],
                                         start=(c == 0), stop=(c == KN - 1))
            for j in range(DJ):
                nc.scalar.copy(acc_sb[:, j * 512:(j + 1) * 512],
                               acc_ps[:, j, :])
                nc.sync.dma_start(acc_out[:, j * 512:(j + 1) * 512],
                                  acc_sb[:, j * 512:(j + 1) * 512])

        nc.vector.reduce_sum(stat[:, 1:2], ssum[:, :NG],
                             axis=mybir.AxisListType.X)
        nc.sync.dma_start(s_out[:], stat[:, 1:2])
        nc.sync.dma_start(m_out[:], stat[:, 0:1])

    nc.compile()
    return nc


def _get_nc():
    if "nc" not in _CACHED:
        _CACHED["nc"] = _build_nc()
    return _CACHED["nc"]


def kernel(x, train, alphas_cumprod, t, **_unused):
    from concourse.bass_utils import run_bass_kernel_spmd

    x = np.asarray(x)
    train = np.asarray(train)
    alphas_cumprod = np.asarray(alphas_cumprod)
    t = np.asarray(t).astype(np.int64)

    xf = x.reshape(B, -1).astype(np.float32)
    tf = train.reshape(N, -1).astype(np.float32)

    acp_t = alphas_cumprod.astype(np.float64)[t]
    a = np.sqrt(acp_t)
    om = 1.0 - acp_t
    gp32 = (a * a / (2.0 * om)).astype(np.float32)   # softmax scale on s''
    xscale = (2.0 / a).astype(np.float32)            # fold into x

    trsq_full = np.einsum("nd,nd->n", tf.astype(np.float64),
                          tf.astype(np.float64)).astype(np.float32)

    t16 = tf.astype(np.float16)
    x16T = np.ascontiguousarray(
        (xscale[:, None] * xf).astype(np.float16).T)  # [D, B]
    ident = np.eye(B, dtype=np.float16)
    g_col = gp32.reshape(B, 1)

    in_maps = []
    for c in range(NCORES):
        sl = slice(c * N_LOC, (c + 1) * N_LOC)
        nat = np.zeros((N_PAD, D), dtype=np.float16)
        nat[:N_LOC] = t16[sl]
        tTc = np.zeros((D, N_PAD), dtype=np.float16)
        tTc[:, :N_LOC] = t16[sl].T
        trsq_c = np.full((N_PAD,), PAD_TRSQ, dtype=np.float32)
        trsq_c[:N_LOC] = trsq_full[sl]
        trsq_c = np.ascontiguousarray(
            np.broadcast_to(trsq_c[None, :], (B, N_PAD)))
        in_maps.append({
            "tT": tTc,
            "tn": nat,
            "xT": x16T,
            "ident": ident,
            "trsq": trsq_c,
            "gcol": g_col,
        })

    nc = _get_nc()
    res = run_bass_kernel_spmd(nc, in_maps, list(range(NCORES)))
    _CACHED["last_results"] = res

    # --- host-side online-softmax merge across cores (fp64) ---
    g64 = gp32.astype(np.float64)
    Ms = np.stack([res.results[c]["m_out"][:, 0].astype(np.float64)
                   for c in range(NCORES)])          # [C, B]
    Ss = np.stack([res.results[c]["s_out"][:, 0].astype(np.float64)
                   for c in range(NCORES)])          # [C, B]
    ACCs = np.stack([res.results[c]["acc_out"].astype(np.float64)
                     for c in range(NCORES)])        # [C, B, D]
    Mg = Ms.max(axis=0)                              # [B]
    scale = np.exp(g64[None, :] * (Ms - Mg[None, :]))  # [C, B]
    den = (scale * Ss).sum(axis=0)                   # [B]
    num = (scale[:, :, None] * ACCs).sum(axis=0)     # [B, D]
    weighted = num / den[:, None]

    coef_x = 1.0 / np.sqrt(om)
    coef_x_hat = a / np.sqrt(om)
    out = coef_x[:, None] * xf.astype(np.float64) - coef_x_hat[:, None] * weighted
    return out.reshape(x.shape).astype(np.float32)



# revision 3
# speedup vs baseline: 1.0574x; 1.0574x over previous
"""Softmax-weighted nearest-neighbor aggregation (DiffusionStar) on 8 TRN2 cores.

Strategy:
  - Shard the train set (N=50000) across 8 cores (6250 rows each, padded to 6272).
  - Per core, two-phase softmax:
      Phase 1 (fp16, DMA ~38.5 MB): scores s[b,n] = (2/a_b)<x_b, t_n> - ||t_n||^2
               via fp16 GEMM streaming a host-pretransposed fp16 train slice;
               ||t||^2 subtracted in fp32 on DVE, per-group row-max tracked.
      Phase 2 (fp8 e3m4, DMA ~19.3 MB): p = 8*exp(g'*(s - M)) on ACT (fp16 out,
               ln8 folded into the bias so p in (0,8] stays out of e3m4
               subnormals), p quantized to e3m4 via the DVE copies (both the
               transposed GEMM operand and an exported row copy), then
               ACC = p8 @ train8 as an e3m4 x e3m4 GEMM streaming the
               natural-layout e3m4 slice (fp32 PSUM accumulate, 49 n-chunks).
  - Host merges (M, S, ACC) across cores with the online-softmax combine and
    applies an exact top-8 residual correction: using the exported quantized
    p8 row, the few dominant neighbors' contributions are replaced by
    p8 * (t_fp32 - t_e3m4), cancelling the e3m4 quantization error where the
    softmax is sharp (which is where coef_x_hat amplifies it).

Numerics: validated ~9e-4 (abs-max relative) vs the fp32 reference in host
simulation; fp16 scores keep the softmax argmax exact, e3m4 only touches the
weighted-sum operand whose error the top-8 correction removes.
"""

import numpy as np

B = 64
D = 3072
N = 50000
NCORES = 8
N_LOC = N // NCORES          # 6250
N_PAD = 6272                 # 49 * 128
KD = D // 128                # 24
KN = N_PAD // 128            # 49
DJ = D // 512                # 6
GROUPS = [(i * 512, 512) for i in range(12)] + [(6144, 128)]
PAD_TRSQ = 1e9
NAT_BUFS = 8
LN_PSCALE = float(np.log(8.0))
TOPK = 8

_CACHED = {}


def _build_nc():
    import concourse.bacc as bacc
    import concourse.tile as tile
    from concourse import mybir
    from contextlib import ExitStack

    f16 = mybir.dt.float16
    f32 = mybir.dt.float32
    f8 = mybir.dt.float8e3

    nc = bacc.Bacc("TRN2", target_bir_lowering=False, debug=False)

    tT = nc.dram_tensor("tT", [D, N_PAD], f16, kind="ExternalInput").ap()
    tn = nc.dram_tensor("tn", [N_PAD, D], f8, kind="ExternalInput").ap()
    xT = nc.dram_tensor("xT", [D, B], f16, kind="ExternalInput").ap()
    ident = nc.dram_tensor("ident", [B, B], f16, kind="ExternalInput").ap()
    trsq = nc.dram_tensor("trsq", [B, N_PAD], f32, kind="ExternalInput").ap()
    gcol = nc.dram_tensor("gcol", [B, 1], f32, kind="ExternalInput").ap()

    acc_out = nc.dram_tensor("acc_out", [B, D], f32, kind="ExternalOutput").ap()
    s_out = nc.dram_tensor("s_out", [B, 1], f32, kind="ExternalOutput").ap()
    m_out = nc.dram_tensor("m_out", [B, 1], f32, kind="ExternalOutput").ap()
    p_out = nc.dram_tensor("p_out", [B, N_PAD], f8, kind="ExternalOutput").ap()

    NG = len(GROUPS)

    with tile.TileContext(nc) as tc, ExitStack() as ctx:
        const = ctx.enter_context(tc.tile_pool(name="const", bufs=1))
        kTp = ctx.enter_context(tc.tile_pool(name="kT", bufs=2))
        natp = ctx.enter_context(tc.tile_pool(name="nat", bufs=NAT_BUFS))
        sb = ctx.enter_context(tc.tile_pool(name="sb", bufs=1))

        # --- constants ---
        xT_sb = const.tile([128, KD, B], f16)
        nc.sync.dma_start(xT_sb[:], xT.rearrange("(k p) b -> p k b", p=128))
        id_sb = const.tile([B, B], f16)
        nc.sync.dma_start(id_sb[:], ident[:])
        trsq_sb = const.tile([B, N_PAD], f32)
        nc.sync.dma_start(trsq_sb[:], trsq[:])
        g_sb = const.tile([B, 1], f32)
        nc.sync.dma_start(g_sb[:], gcol[:])

        mpart = sb.tile([B, NG], f32)
        ssum = sb.tile([B, NG], f32)
        stat = sb.tile([B, 4], f32)
        acc_sb = sb.tile([B, D], f32)
        p8row = sb.tile([B, N_PAD], f8)
        sc_tiles = []

        tTr = tT.rearrange("(k p) n -> p k n", p=128)  # [128, KD, N_PAD]

        # --- phase 1: scores + per-group max ---
        with tc.tile_pool(name="psS", bufs=2, space="PSUM") as psS:
            for gi, (n0, W) in enumerate(GROUPS):
                kT = kTp.tile([128, KD, 512], f16, tag="kT")
                nc.sync.dma_start(kT[:, :, :W], tTr[:, :, n0:n0 + W])
                ps = psS.tile([B, 512], f32, tag="ps")
                for k in range(KD):
                    nc.tensor.matmul(ps[:, :W], xT_sb[:, k, :], kT[:, k, :W],
                                     start=(k == 0), stop=(k == KD - 1))
                sc = sb.tile([B, 512], f32, tag=f"sc{gi}")
                sc_tiles.append(sc)
                nc.vector.tensor_tensor(sc[:, :W], ps[:, :W],
                                        trsq_sb[:, n0:n0 + W],
                                        op=mybir.AluOpType.subtract)
                nc.vector.reduce_max(mpart[:, gi:gi + 1], sc[:, :W],
                                     axis=mybir.AxisListType.X)

        # --- global max, bias = -g*M + ln(8) ---
        nc.vector.reduce_max(stat[:, 0:1], mpart[:, :NG],
                             axis=mybir.AxisListType.X)
        nc.vector.tensor_tensor(stat[:, 2:3], g_sb[:], stat[:, 0:1],
                                op=mybir.AluOpType.mult)
        nc.vector.tensor_scalar_mul(stat[:, 2:3], stat[:, 2:3], -1.0)
        nc.vector.tensor_scalar_add(stat[:, 2:3], stat[:, 2:3], LN_PSCALE)

        # --- exp -> (e3m4 row copy, transpose) -> GEMM2, pipelined per group ---
        with tc.tile_pool(name="psT", bufs=2, space="PSUM") as psT, \
             tc.tile_pool(name="psA", bufs=1, space="PSUM") as psA:
            acc_ps = psA.tile([B, DJ, 512], f32)
            for gi, (n0, W) in enumerate(GROUPS):
                p = sb.tile([B, 512], f16, tag=f"p{gi}")
                nc.scalar.activation(p[:, :W], sc_tiles[gi][:, :W],
                                     mybir.ActivationFunctionType.Exp,
                                     bias=stat[:, 2:3], scale=g_sb[:],
                                     accum_out=ssum[:, gi:gi + 1])
                nc.vector.tensor_copy(p8row[:, n0:n0 + W], p[:, :W])
                for ci in range(W // 128):
                    c = n0 // 128 + ci
                    pt_ps = psT.tile([128, B], f16, tag="pt")
                    nc.tensor.transpose(pt_ps[:],
                                        p[:, ci * 128:(ci + 1) * 128],
                                        id_sb[:])
                    pT = sb.tile([128, B], f8, tag=f"pT{c}")
                    nc.vector.tensor_copy(pT[:], pt_ps[:])
                    nat = natp.tile([128, D], f8, tag="nat")
                    nc.sync.dma_start(nat[:], tn[c * 128:(c + 1) * 128, :])
                    for j in range(DJ):
                        nc.tensor.matmul(acc_ps[:, j, :], pT[:],
                                         nat[:, j * 512:(j + 1) * 512],
                                         start=(c == 0), stop=(c == KN - 1))
            for j in range(DJ):
                nc.scalar.copy(acc_sb[:, j * 512:(j + 1) * 512],
                               acc_ps[:, j, :])
                nc.sync.dma_start(acc_out[:, j * 512:(j + 1) * 512],
                                  acc_sb[:, j * 512:(j + 1) * 512])

        nc.vector.reduce_sum(stat[:, 1:2], ssum[:, :NG],
                             axis=mybir.AxisListType.X)
        nc.sync.dma_start(s_out[:], stat[:, 1:2])
        nc.sync.dma_start(m_out[:], stat[:, 0:1])
        nc.sync.dma_start(p_out[:], p8row[:])

    nc.compile()
    return nc


def _get_nc():
    if "nc" not in _CACHED:
        _CACHED["nc"] = _build_nc()
    return _CACHED["nc"]


def kernel(x, train, alphas_cumprod, t, **_unused):
    import ml_dtypes
    from concourse.bass_utils import run_bass_kernel_spmd

    e3 = ml_dtypes.float8_e3m4

    x = np.asarray(x)
    train = np.asarray(train)
    alphas_cumprod = np.asarray(alphas_cumprod)
    t = np.asarray(t).astype(np.int64)

    xf = x.reshape(B, -1).astype(np.float32)
    tf = train.reshape(N, -1).astype(np.float32)

    acp_t = alphas_cumprod.astype(np.float64)[t]
    a = np.sqrt(acp_t)
    om = 1.0 - acp_t
    gp32 = (a * a / (2.0 * om)).astype(np.float32)   # softmax scale on s
    xscale = (2.0 / a).astype(np.float32)            # fold into x

    trsq_full = np.einsum("nd,nd->n", tf.astype(np.float64),
                          tf.astype(np.float64)).astype(np.float32)

    t16 = tf.astype(np.float16)
    t8 = tf.astype(e3)                               # GEMM2 operand (and its
    t8f = t8.astype(np.float32)                      # host-side exact value)
    x16T = np.ascontiguousarray(
        (xscale[:, None] * xf).astype(np.float16).T)  # [D, B]
    ident = np.eye(B, dtype=np.float16)
    g_col = gp32.reshape(B, 1)

    in_maps = []
    for c in range(NCORES):
        sl = slice(c * N_LOC, (c + 1) * N_LOC)
        nat = np.zeros((N_PAD, D), dtype=e3)
        nat[:N_LOC] = t8[sl]
        tTc = np.zeros((D, N_PAD), dtype=np.float16)
        tTc[:, :N_LOC] = t16[sl].T
        trsq_c = np.full((N_PAD,), PAD_TRSQ, dtype=np.float32)
        trsq_c[:N_LOC] = trsq_full[sl]
        trsq_c = np.ascontiguousarray(
            np.broadcast_to(trsq_c[None, :], (B, N_PAD)))
        in_maps.append({
            "tT": tTc,
            "tn": nat,
            "xT": x16T,
            "ident": ident,
            "trsq": trsq_c,
            "gcol": g_col,
        })

    nc = _get_nc()
    res = run_bass_kernel_spmd(nc, in_maps, list(range(NCORES)))
    _CACHED["last_results"] = res

    # --- host-side online-softmax merge across cores (fp64) ---
    g64 = gp32.astype(np.float64)
    Ms = np.stack([res.results[c]["m_out"][:, 0].astype(np.float64)
                   for c in range(NCORES)])          # [C, B]
    Ss = np.stack([res.results[c]["s_out"][:, 0].astype(np.float64)
                   for c in range(NCORES)])          # [C, B]
    ACCs = np.stack([res.results[c]["acc_out"].astype(np.float64)
                     for c in range(NCORES)])        # [C, B, D]

    # --- exact top-K e3m4 residual correction per core ---
    bidx = np.arange(B)
    for c in range(NCORES):
        p8 = np.asarray(res.results[c]["p_out"]).view(e3).astype(np.float32)
        idx = np.argpartition(-p8, TOPK, axis=1)[:, :TOPK]   # [B, K] local n
        w = np.take_along_axis(p8, idx, axis=1).astype(np.float64)
        idx = np.minimum(idx, N_LOC - 1)   # pads only selected when w == 0
        gidx = idx + c * N_LOC
        resid = (tf[gidx].astype(np.float64)
                 - t8f[gidx].astype(np.float64))             # [B, K, D]
        ACCs[c] += np.einsum("bk,bkd->bd", w, resid)

    Mg = Ms.max(axis=0)                              # [B]
    scale = np.exp(g64[None, :] * (Ms - Mg[None, :]))  # [C, B]
    den = (scale * Ss).sum(axis=0)                   # [B]
    num = (scale[:, :, None] * ACCs).sum(axis=0)     # [C sum -> B, D]
    weighted = num / den[:, None]

    coef_x = 1.0 / np.sqrt(om)
    coef_x_hat = a / np.sqrt(om)
    out = coef_x[:, None] * xf.astype(np.float64) - coef_x_hat[:, None] * weighted
    return out.reshape(x.shape).astype(np.float32)


# revision 9
# speedup vs baseline: 1.4382x; 1.3601x over previous
"""Softmax-weighted nearest-neighbor aggregation (DiffusionStar) on 8 TRN2 cores.

Strategy:
  - Shard the train set (N=50000) across 8 cores (6250 rows each, padded to 6272).
  - All train data streams as fp8 e3m4 (~19.3 MB per copy per core); scores:
      Phase 1: sc[b,n] = <x8_b, t8_n> - (a_b/2)||t_n||^2 via e3m4 GEMM
               (fp32 PSUM) minus a host-prescaled fp32 row; per-group row-max.
      Phase 2: p = 8*exp(gamma*(sc - M)) on ACT (f16 out, ln8 in the bias keeps
               p in (0,8] clear of e3m4 subnormals), p quantized to e3m4 (both
               the transposed GEMM operand and an exported row copy), then
               ACC = p8 @ t8 as an e3m4 GEMM (fp32 PSUM, 49 n-chunks).
  - DRAM layouts are host-pretiled so each dma_start lands as one long
    contiguous run per partition (24 KB for the score stream, 12 KB for the
    natural stream) - descriptor-amortized, near-peak HBM rate.
  - Host merge (fp64): per-core exact top-8 rescore - using the exported p8
    row, the 8 dominant candidates' contributions (weights AND train rows) are
    replaced with exact fp64 values; then the standard online-softmax combine
    across cores. This cancels the fp8 noise exactly where the softmax is
    sharp; the diffuse tail averages it out. Validated ~4e-5 in host sim.
"""

import numpy as np

B = 64
D = 3072
N = 50000
NCORES = 8
N_LOC = N // NCORES          # 6250
N_PAD = 6272                 # 49 * 128
KD = D // 128                # 24
KN = N_PAD // 128            # 49
DJ = D // 512                # 6
NGF = 12                     # full 512-wide groups; last group is 128 wide
GROUPS = [(i * 512, 512) for i in range(NGF)] + [(6144, 128)]
PAD_TRSQ = 1e9
LN_PSCALE = float(np.log(8.0))
TOPK = 8
NAT_Q = 4                    # chunks per natural-stream DMA
NAT_BUFS = 3

_CACHED = {}


def _build_nc():
    import concourse.bacc as bacc
    import concourse.tile as tile
    from concourse import mybir
    from contextlib import ExitStack

    f16 = mybir.dt.float16
    f32 = mybir.dt.float32
    f8 = mybir.dt.float8e3

    nc = bacc.Bacc("TRN2", target_bir_lowering=False, debug=False)

    # host-pretiled DRAM layouts (partition-major, long contiguous runs)
    tTg = nc.dram_tensor("tTg", [128, NGF, KD, 512], f8, kind="ExternalInput").ap()
    tTl = nc.dram_tensor("tTl", [128, KD, 128], f8, kind="ExternalInput").ap()
    natq = nc.dram_tensor("natq", [128, KN, D], f8, kind="ExternalInput").ap()
    xT = nc.dram_tensor("xT", [128, KD, B], f8, kind="ExternalInput").ap()
    ident = nc.dram_tensor("ident", [B, B], f16, kind="ExternalInput").ap()
    trsq = nc.dram_tensor("trsq", [B, N_PAD], f32, kind="ExternalInput").ap()
    gcol = nc.dram_tensor("gcol", [B, 1], f32, kind="ExternalInput").ap()

    acc_out = nc.dram_tensor("acc_out", [B, D], f32, kind="ExternalOutput").ap()
    m_out = nc.dram_tensor("m_out", [B, 1], f32, kind="ExternalOutput").ap()
    p_out = nc.dram_tensor("p_out", [B, N_PAD], f8, kind="ExternalOutput").ap()

    NG = len(GROUPS)

    with tile.TileContext(nc) as tc, ExitStack() as ctx:
        const = ctx.enter_context(tc.tile_pool(name="const", bufs=1))
        kTp = ctx.enter_context(tc.tile_pool(name="kT", bufs=2))
        natp = ctx.enter_context(tc.tile_pool(name="nat", bufs=NAT_BUFS))
        sb = ctx.enter_context(tc.tile_pool(name="sb", bufs=1))

        # --- constants ---
        xT_sb = const.tile([128, KD, B], f8)
        nc.sync.dma_start(xT_sb[:], xT[:])
        id_sb = const.tile([B, B], f16)
        nc.sync.dma_start(id_sb[:], ident[:])
        trsq_sb = const.tile([B, N_PAD], f32)
        nc.sync.dma_start(trsq_sb[:], trsq[:])
        g_sb = const.tile([B, 1], f32)
        nc.sync.dma_start(g_sb[:], gcol[:])

        mpart = sb.tile([B, NG], f32)
        stat = sb.tile([B, 4], f32)
        acc_sb = sb.tile([B, D], f32)
        p8row = sb.tile([B, N_PAD], f8)
        sc_tiles = []

        # --- phase 1: scores + per-group max (groups loaded in pairs) ---
        with tc.tile_pool(name="psS", bufs=2, space="PSUM") as psS:
            for gp in range(NGF // 2):
                kT = kTp.tile([128, 2, KD, 512], f8, tag="kT")
                nc.sync.dma_start(kT[:], tTg[:, 2 * gp:2 * gp + 2])
                for gg in range(2):
                    gi = 2 * gp + gg
                    n0 = gi * 512
                    ps = psS.tile([B, 512], f32, tag="ps")
                    for k in range(KD):
                        nc.tensor.matmul(ps[:], xT_sb[:, k, :],
                                         kT[:, gg, k, :],
                                         start=(k == 0), stop=(k == KD - 1))
                    sc = sb.tile([B, 512], f32, tag=f"sc{gi}")
                    sc_tiles.append(sc)
                    nc.vector.tensor_tensor(sc[:], ps[:],
                                            trsq_sb[:, n0:n0 + 512],
                                            op=mybir.AluOpType.subtract)
                    nc.vector.reduce_max(mpart[:, gi:gi + 1], sc[:],
                                         axis=mybir.AxisListType.X)
            # last 128-wide group
            kTe = kTp.tile([128, KD, 128], f8, tag="kTe")
            nc.sync.dma_start(kTe[:], tTl[:])
            ps = psS.tile([B, 512], f32, tag="ps")
            for k in range(KD):
                nc.tensor.matmul(ps[:, :128], xT_sb[:, k, :], kTe[:, k, :],
                                 start=(k == 0), stop=(k == KD - 1))
            sc = sb.tile([B, 512], f32, tag=f"sc{NG - 1}")
            sc_tiles.append(sc)
            nc.vector.tensor_tensor(sc[:, :128], ps[:, :128],
                                    trsq_sb[:, 6144:6272],
                                    op=mybir.AluOpType.subtract)
            nc.vector.reduce_max(mpart[:, NG - 1:NG], sc[:, :128],
                                 axis=mybir.AxisListType.X)

        # --- global max, bias = -g*M + ln(8) ---
        nc.vector.reduce_max(stat[:, 0:1], mpart[:, :NG],
                             axis=mybir.AxisListType.X)
        nc.vector.tensor_tensor(stat[:, 2:3], g_sb[:], stat[:, 0:1],
                                op=mybir.AluOpType.mult)
        nc.vector.tensor_scalar_mul(stat[:, 2:3], stat[:, 2:3], -1.0)
        nc.vector.tensor_scalar_add(stat[:, 2:3], stat[:, 2:3], LN_PSCALE)

        # --- exp -> (e3m4 row copy, transpose) -> GEMM2, pipelined ---
        with tc.tile_pool(name="psT", bufs=2, space="PSUM") as psT, \
             tc.tile_pool(name="psA", bufs=1, space="PSUM") as psA:
            acc_ps = psA.tile([B, DJ, 512], f32)
            p_tiles = []
            for gi, (n0, W) in enumerate(GROUPS):
                p = sb.tile([B, 512], f16, tag=f"p{gi}")
                p_tiles.append(p)
                nc.scalar.activation(p[:, :W], sc_tiles[gi][:, :W],
                                     mybir.ActivationFunctionType.Exp,
                                     bias=stat[:, 2:3], scale=g_sb[:])
                nc.vector.tensor_copy(p8row[:, n0:n0 + W], p[:, :W])
            nat_tiles = [None] * KN
            for c in range(KN):
                if c % NAT_Q == 0:
                    nq = min(NAT_Q, KN - c)
                    natt = natp.tile([128, NAT_Q, D], f8, tag="nat")
                    nc.sync.dma_start(natt[:, :nq, :], natq[:, c:c + nq, :])
                    for i in range(nq):
                        nat_tiles[c + i] = natt[:, i, :]
                gi, ci = c // 4, c % 4
                pt_ps = psT.tile([128, B], f16, tag="pt")
                nc.tensor.transpose(pt_ps[:],
                                    p_tiles[gi][:, ci * 128:(ci + 1) * 128],
                                    id_sb[:])
                pT = sb.tile([128, B], f8, tag=f"pT{c}")
                nc.vector.tensor_copy(pT[:], pt_ps[:])
                nat = nat_tiles[c]
                for j in range(DJ):
                    nc.tensor.matmul(acc_ps[:, j, :], pT[:],
                                     nat[:, j * 512:(j + 1) * 512],
                                     start=(c == 0), stop=(c == KN - 1))
            for j in range(DJ):
                nc.scalar.copy(acc_sb[:, j * 512:(j + 1) * 512],
                               acc_ps[:, j, :])
                nc.sync.dma_start(acc_out[:, j * 512:(j + 1) * 512],
                                  acc_sb[:, j * 512:(j + 1) * 512])

        nc.sync.dma_start(m_out[:], stat[:, 0:1])
        nc.sync.dma_start(p_out[:], p8row[:])

    nc.compile()
    return nc


def _get_nc():
    if "nc" not in _CACHED:
        _CACHED["nc"] = _build_nc()
    return _CACHED["nc"]


def kernel(x, train, alphas_cumprod, t, **_unused):
    import ml_dtypes
    from concourse.bass_utils import run_bass_kernel_spmd

    e3 = ml_dtypes.float8_e3m4

    x = np.asarray(x)
    train = np.asarray(train)
    alphas_cumprod = np.asarray(alphas_cumprod)
    t = np.asarray(t).astype(np.int64)

    xf = x.reshape(B, -1).astype(np.float32)
    tf = train.reshape(N, -1).astype(np.float32)

    acp_t = alphas_cumprod.astype(np.float64)[t]
    a = np.sqrt(acp_t)
    om = 1.0 - acp_t
    g64 = a / om                                     # softmax scale on sc
    gp32 = g64.astype(np.float32)

    trsq_full = np.einsum("nd,nd->n", tf.astype(np.float64),
                          tf.astype(np.float64))

    t8 = tf.astype(e3)
    t8f = t8.astype(np.float32)
    x8 = xf.astype(e3)                               # |x| <= ~4.5, in range
    xTq = np.ascontiguousarray(
        x8.T.reshape(KD, 128, B).transpose(1, 0, 2))  # [128, KD, B]
    ident = np.eye(B, dtype=np.float16)
    g_col = gp32.reshape(B, 1)

    in_maps = []
    for c in range(NCORES):
        sl = slice(c * N_LOC, (c + 1) * N_LOC)
        t8c = np.zeros((N_PAD, D), dtype=e3)
        t8c[:N_LOC] = t8[sl]
        natq_c = np.ascontiguousarray(
            t8c.reshape(KN, 128, D).transpose(1, 0, 2))       # [128, KN, D]
        tTg_c = np.ascontiguousarray(
            t8c[:NGF * 512].reshape(NGF, 512, KD, 128)
            .transpose(3, 0, 2, 1))                  # [128, NGF, KD, 512]
        tTl_c = np.ascontiguousarray(
            t8c[NGF * 512:].reshape(128, KD, 128)
            .transpose(2, 1, 0))                     # [128, KD, 128]
        trsq_c = np.full((N_PAD,), PAD_TRSQ, dtype=np.float64)
        trsq_c[:N_LOC] = trsq_full[sl]
        trsq_c = ((a[:, None] / 2.0) * trsq_c[None, :]).astype(np.float32)
        in_maps.append({
            "tTg": tTg_c,
            "tTl": tTl_c,
            "natq": natq_c,
            "xT": xTq,
            "ident": ident,
            "trsq": np.ascontiguousarray(trsq_c),
            "gcol": g_col,
        })

    nc = _get_nc()
    res = run_bass_kernel_spmd(nc, in_maps, list(range(NCORES)))
    _CACHED["last_results"] = res

    # --- host merge: exact top-K rescore per core + online-softmax combine ---
    xf64 = xf.astype(np.float64)
    stats = []
    for c in range(NCORES):
        M = res.results[c]["m_out"][:, 0].astype(np.float64)
        ACC = res.results[c]["acc_out"].astype(np.float64)
        p8 = np.asarray(res.results[c]["p_out"]).view(e3).astype(np.float32)
        S = p8.astype(np.float64).sum(axis=1)   # consistent with ACC's p8
        idx = np.argpartition(-p8, TOPK, axis=1)[:, :TOPK]
        pq = np.take_along_axis(p8, idx, axis=1).astype(np.float64)
        idx = np.minimum(idx, N_LOC - 1)   # pads only selected when pq == 0
        gidx = idx + c * N_LOC
        tr_top = tf[gidx].astype(np.float64)                  # [B, K, D]
        sc_exact = (np.einsum("bkd,bd->bk", tr_top, xf64)
                    - (a[:, None] / 2.0) * trsq_full[gidx])
        Mstar = np.maximum(M, sc_exact.max(axis=1))
        shift = np.exp(g64 * (M - Mstar))
        pstar = 8.0 * np.exp(g64[:, None] * (sc_exact - Mstar[:, None]))
        S = S * shift - (pq * shift[:, None]).sum(axis=1) + pstar.sum(axis=1)
        ACC = ACC * shift[:, None] \
            - np.einsum("bk,bkd->bd", pq * shift[:, None],
                        t8f[gidx].astype(np.float64)) \
            + np.einsum("bk,bkd->bd", pstar, tr_top)
        stats.append((Mstar, S, ACC))

    Mg = np.max(np.stack([s[0] for s in stats]), axis=0)
    den = np.zeros(B)
    num = np.zeros((B, D))
    for Mc, S, ACC in stats:
        sl = np.exp(g64 * (Mc - Mg))
        den += sl * S
        num += sl[:, None] * ACC
    weighted = num / den[:, None]

    coef_x = 1.0 / np.sqrt(om)
    coef_x_hat = a / np.sqrt(om)
    out = coef_x[:, None] * xf64 - coef_x_hat[:, None] * weighted
    return out.reshape(x.shape).astype(np.float32)


# revision 15
# speedup vs baseline: 1.5115x; 1.0510x over previous
"""Softmax-weighted nearest-neighbor aggregation (DiffusionStar) on 8 TRN2 cores.

Strategy:
  - Shard the train set (N=50000) across 8 cores (6250 rows each, padded to 6272).
  - All train data streams as fp8 e3m4 (~19.3 MB per copy per core); scores:
      Phase 1: sc[b,n] = <x8_b, t8_n> - (a_b/2)||t_n||^2 via e3m4 GEMM
               (fp32 PSUM) minus a host-prescaled fp32 row; per-group row-max.
      Phase 2: p = 8*exp(gamma*(sc - M)) on ACT (f16 out, ln8 in the bias keeps
               p in (0,8] clear of e3m4 subnormals), p quantized to e3m4 (both
               the transposed GEMM operand and an exported row copy), then
               ACC = p8 @ t8 as an e3m4 GEMM (fp32 PSUM, 49 n-chunks).
  - DRAM layouts are host-pretiled so each dma_start lands as one long
    contiguous run per partition (24 KB for the score stream, 12 KB for the
    natural stream) - descriptor-amortized, near-peak HBM rate.
  - Host merge (fp64): per-core exact top-8 rescore - using the exported p8
    row, the 8 dominant candidates' contributions (weights AND train rows) are
    replaced with exact fp64 values; then the standard online-softmax combine
    across cores. This cancels the fp8 noise exactly where the softmax is
    sharp; the diffuse tail averages it out. Validated ~4e-5 in host sim.
"""

import numpy as np

B = 64
D = 3072
N = 50000
NCORES = 8
N_LOC = N // NCORES          # 6250
N_PAD = 6272                 # 49 * 128
KD = D // 128                # 24
KN = N_PAD // 128            # 49
DJ = D // 512                # 6
NGF = 12                     # full 512-wide groups; last group is 128 wide
GROUPS = [(i * 512, 512) for i in range(NGF)] + [(6144, 128)]
PAD_TRSQ = 1e9
LN_PSCALE = float(np.log(8.0))
TOPK = 8
NAT_Q = 4                    # chunks per natural-stream DMA
NAT_BUFS = 3

_CACHED = {}


def _build_nc():
    import concourse.bacc as bacc
    import concourse.tile as tile
    from concourse import mybir
    from contextlib import ExitStack

    f16 = mybir.dt.float16
    f32 = mybir.dt.float32
    f8 = mybir.dt.float8e3

    nc = bacc.Bacc("TRN2", target_bir_lowering=False, debug=False)

    # host-pretiled DRAM layouts (partition-major, long contiguous runs)
    tTg = nc.dram_tensor("tTg", [128, NGF, KD, 512], f8, kind="ExternalInput").ap()
    tTl = nc.dram_tensor("tTl", [128, KD, 128], f8, kind="ExternalInput").ap()
    natq = nc.dram_tensor("natq", [128, KN, D], f8, kind="ExternalInput").ap()
    xT = nc.dram_tensor("xT", [128, KD, B], f8, kind="ExternalInput").ap()
    ident = nc.dram_tensor("ident", [B, B], f16, kind="ExternalInput").ap()
    trsq = nc.dram_tensor("trsq", [B, N_PAD], f32, kind="ExternalInput").ap()
    gcol = nc.dram_tensor("gcol", [B, 1], f32, kind="ExternalInput").ap()

    acc_out = nc.dram_tensor("acc_out", [B, D], f32, kind="ExternalOutput").ap()
    m_out = nc.dram_tensor("m_out", [B, 1], f32, kind="ExternalOutput").ap()
    p_out = nc.dram_tensor("p_out", [B, N_PAD], f8, kind="ExternalOutput").ap()

    NG = len(GROUPS)

    with tile.TileContext(nc) as tc, ExitStack() as ctx:
        const = ctx.enter_context(tc.tile_pool(name="const", bufs=1))
        kTp = ctx.enter_context(tc.tile_pool(name="kT", bufs=2))
        natp = ctx.enter_context(tc.tile_pool(name="nat", bufs=NAT_BUFS))
        sb = ctx.enter_context(tc.tile_pool(name="sb", bufs=1))
        hip = ctx.enter_context(tc.tile_pool(name="hi", bufs=3))

        # --- constants ---
        xT_sb = const.tile([128, KD, B], f8)
        nc.sync.dma_start(xT_sb[:], xT[:])
        id_sb = const.tile([B, B], f16)
        nc.sync.dma_start(id_sb[:], ident[:])
        trsq_sb = const.tile([B, N_PAD], f32)
        nc.sync.dma_start(trsq_sb[:], trsq[:])
        g_sb = const.tile([B, 1], f32)
        nc.sync.dma_start(g_sb[:], gcol[:])

        mpart = sb.tile([B, NG], f32)
        stat = sb.tile([B, 4], f32)
        acc_sb = sb.tile([B, D], f32)
        p8row = sb.tile([B, N_PAD], f8)
        sc_tiles = []

        # --- phase 1: scores + per-group max (groups loaded in pairs).
        #     Col-tiled: even k-chunks accumulate on PSUM partitions 0-63,
        #     odd k-chunks on 64-127, concurrently in separate array halves;
        #     DVE folds the two partials while subtracting ||t||^2. ---
        KH = KD // 2
        with tc.tile_pool(name="psS", bufs=2, space="PSUM") as psS:
            for gp in range(NGF // 2):
                kT = kTp.tile([128, 2, KD, 512], f8, tag="kT")
                nc.sync.dma_start(kT[:], tTg[:, 2 * gp:2 * gp + 2])
                for gg in range(2):
                    gi = 2 * gp + gg
                    n0 = gi * 512
                    ps = psS.tile([128, 512], f32, tag="ps")
                    for kk in range(KH):
                        nc.tensor.matmul(ps[0:B, :], xT_sb[:, 2 * kk, :],
                                         kT[:, gg, 2 * kk, :],
                                         start=(kk == 0), stop=(kk == KH - 1))
                        nc.tensor.matmul(ps[B:128, :], xT_sb[:, 2 * kk + 1, :],
                                         kT[:, gg, 2 * kk + 1, :],
                                         start=(kk == 0), stop=(kk == KH - 1))
                    sc = sb.tile([B, 512], f32, tag=f"sc{gi}")
                    sc_tiles.append(sc)
                    hi = hip.tile([128, 512], f32, tag="hi")
                    nc.scalar.copy(hi[B:128, :], ps[B:128, :])
                    nc.vector.tensor_tensor(sc[:], ps[0:B, :], hi[B:128, :],
                                            op=mybir.AluOpType.add)
                    nc.vector.tensor_tensor(sc[:], sc[:],
                                            trsq_sb[:, n0:n0 + 512],
                                            op=mybir.AluOpType.subtract)
                    nc.vector.reduce_max(mpart[:, gi:gi + 1], sc[:],
                                         axis=mybir.AxisListType.X)
            # last 128-wide group
            kTe = kTp.tile([128, KD, 128], f8, tag="kTe")
            nc.sync.dma_start(kTe[:], tTl[:])
            ps = psS.tile([128, 512], f32, tag="ps")
            for kk in range(KH):
                nc.tensor.matmul(ps[0:B, :128], xT_sb[:, 2 * kk, :],
                                 kTe[:, 2 * kk, :],
                                 start=(kk == 0), stop=(kk == KH - 1))
                nc.tensor.matmul(ps[B:128, :128], xT_sb[:, 2 * kk + 1, :],
                                 kTe[:, 2 * kk + 1, :],
                                 start=(kk == 0), stop=(kk == KH - 1))
            sc = sb.tile([B, 512], f32, tag=f"sc{NG - 1}")
            sc_tiles.append(sc)
            hi = hip.tile([128, 512], f32, tag="hi")
            nc.scalar.copy(hi[B:128, :128], ps[B:128, :128])
            nc.vector.tensor_tensor(sc[:, :128], ps[0:B, :128], hi[B:128, :128],
                                    op=mybir.AluOpType.add)
            nc.vector.tensor_tensor(sc[:, :128], sc[:, :128],
                                    trsq_sb[:, 6144:6272],
                                    op=mybir.AluOpType.subtract)
            nc.vector.reduce_max(mpart[:, NG - 1:NG], sc[:, :128],
                                 axis=mybir.AxisListType.X)

        # --- global max, bias = -g*M + ln(8) ---
        nc.vector.reduce_max(stat[:, 0:1], mpart[:, :NG],
                             axis=mybir.AxisListType.X)
        nc.vector.tensor_tensor(stat[:, 2:3], g_sb[:], stat[:, 0:1],
                                op=mybir.AluOpType.mult)
        nc.vector.tensor_scalar_mul(stat[:, 2:3], stat[:, 2:3], -1.0)
        nc.vector.tensor_scalar_add(stat[:, 2:3], stat[:, 2:3], LN_PSCALE)

        # --- exp -> e3m4 row copy -> batched transposes -> col-tiled GEMM2.
        #     Even n-chunks accumulate on PSUM partitions 0-63, odd on
        #     64-127; DVE folds the halves during evacuation. ---
        with tc.tile_pool(name="psT", bufs=2, space="PSUM") as psT, \
             tc.tile_pool(name="psA", bufs=1, space="PSUM") as psA:
            acc_ps = psA.tile([128, DJ, 512], f32)
            pT_tiles = []
            for gi, (n0, W) in enumerate(GROUPS):
                p = sb.tile([B, 512], f16, tag=f"p{gi}")
                nc.scalar.activation(p[:, :W], sc_tiles[gi][:, :W],
                                     mybir.ActivationFunctionType.Exp,
                                     bias=stat[:, 2:3], scale=g_sb[:])
                nc.vector.tensor_copy(p8row[:, n0:n0 + W], p[:, :W])
                for ci in range(W // 128):
                    c = n0 // 128 + ci
                    pt_ps = psT.tile([128, B], f16, tag="pt")
                    nc.tensor.transpose(pt_ps[:],
                                        p[:, ci * 128:(ci + 1) * 128],
                                        id_sb[:])
                    pT = sb.tile([128, B], f8, tag=f"pT{c}")
                    nc.vector.tensor_copy(pT[:], pt_ps[:])
                    pT_tiles.append(pT)
            nat_tiles = [None] * KN
            for c in range(KN):
                if c % NAT_Q == 0:
                    nq = min(NAT_Q, KN - c)
                    natt = natp.tile([128, NAT_Q, D], f8, tag="nat")
                    nc.sync.dma_start(natt[:, :nq, :], natq[:, c:c + nq, :])
                    for i in range(nq):
                        nat_tiles[c + i] = natt[:, i, :]
                half = c % 2
                o0 = half * B
                for j in range(DJ):
                    nc.tensor.matmul(acc_ps[o0:o0 + B, j, :], pT_tiles[c][:],
                                     nat_tiles[c][:, j * 512:(j + 1) * 512],
                                     start=(c == half), stop=(c >= KN - 2))
            acc_hi = sb.tile([128, D], f32)
            for j in range(DJ):
                nc.scalar.copy(acc_hi[B:128, j * 512:(j + 1) * 512],
                               acc_ps[B:128, j, :])
                nc.vector.tensor_tensor(acc_sb[:, j * 512:(j + 1) * 512],
                                        acc_ps[0:B, j, :],
                                        acc_hi[B:128, j * 512:(j + 1) * 512],
                                        op=mybir.AluOpType.add)
                nc.sync.dma_start(acc_out[:, j * 512:(j + 1) * 512],
                                  acc_sb[:, j * 512:(j + 1) * 512])

        nc.sync.dma_start(m_out[:], stat[:, 0:1])
        nc.sync.dma_start(p_out[:], p8row[:])

    nc.compile()
    return nc


def _get_nc():
    if "nc" not in _CACHED:
        _CACHED["nc"] = _build_nc()
    return _CACHED["nc"]


def kernel(x, train, alphas_cumprod, t, **_unused):
    import ml_dtypes
    from concourse.bass_utils import run_bass_kernel_spmd

    e3 = ml_dtypes.float8_e3m4

    x = np.asarray(x)
    train = np.asarray(train)
    alphas_cumprod = np.asarray(alphas_cumprod)
    t = np.asarray(t).astype(np.int64)

    xf = x.reshape(B, -1).astype(np.float32)
    tf = train.reshape(N, -1).astype(np.float32)

    acp_t = alphas_cumprod.astype(np.float64)[t]
    a = np.sqrt(acp_t)
    om = 1.0 - acp_t
    g64 = a / om                                     # softmax scale on sc
    gp32 = g64.astype(np.float32)

    trsq_full = np.einsum("nd,nd->n", tf.astype(np.float64),
                          tf.astype(np.float64))

    t8 = tf.astype(e3)
    t8f = t8.astype(np.float32)
    x8 = xf.astype(e3)                               # |x| <= ~4.5, in range
    xTq = np.ascontiguousarray(
        x8.T.reshape(KD, 128, B).transpose(1, 0, 2))  # [128, KD, B]
    ident = np.eye(B, dtype=np.float16)
    g_col = gp32.reshape(B, 1)

    in_maps = []
    for c in range(NCORES):
        sl = slice(c * N_LOC, (c + 1) * N_LOC)
        t8c = np.zeros((N_PAD, D), dtype=e3)
        t8c[:N_LOC] = t8[sl]
        natq_c = np.ascontiguousarray(
            t8c.reshape(KN, 128, D).transpose(1, 0, 2))       # [128, KN, D]
        tTg_c = np.ascontiguousarray(
            t8c[:NGF * 512].reshape(NGF, 512, KD, 128)
            .transpose(3, 0, 2, 1))                  # [128, NGF, KD, 512]
        tTl_c = np.ascontiguousarray(
            t8c[NGF * 512:].reshape(128, KD, 128)
            .transpose(2, 1, 0))                     # [128, KD, 128]
        trsq_c = np.full((N_PAD,), PAD_TRSQ, dtype=np.float64)
        trsq_c[:N_LOC] = trsq_full[sl]
        trsq_c = ((a[:, None] / 2.0) * trsq_c[None, :]).astype(np.float32)
        in_maps.append({
            "tTg": tTg_c,
            "tTl": tTl_c,
            "natq": natq_c,
            "xT": xTq,
            "ident": ident,
            "trsq": np.ascontiguousarray(trsq_c),
            "gcol": g_col,
        })

    nc = _get_nc()
    res = run_bass_kernel_spmd(nc, in_maps, list(range(NCORES)))
    _CACHED["last_results"] = res

    # --- host merge: exact top-K rescore per core + online-softmax combine ---
    xf64 = xf.astype(np.float64)
    stats = []
    for c in range(NCORES):
        M = res.results[c]["m_out"][:, 0].astype(np.float64)
        ACC = res.results[c]["acc_out"].astype(np.float64)
        p8 = np.asarray(res.results[c]["p_out"]).view(e3).astype(np.float32)
        S = p8.astype(np.float64).sum(axis=1)   # consistent with ACC's p8
        idx = np.argpartition(-p8, TOPK, axis=1)[:, :TOPK]
        pq = np.take_along_axis(p8, idx, axis=1).astype(np.float64)
        idx = np.minimum(idx, N_LOC - 1)   # pads only selected when pq == 0
        gidx = idx + c * N_LOC
        tr_top = tf[gidx].astype(np.float64)                  # [B, K, D]
        sc_exact = (np.einsum("bkd,bd->bk", tr_top, xf64)
                    - (a[:, None] / 2.0) * trsq_full[gidx])
        Mstar = np.maximum(M, sc_exact.max(axis=1))
        shift = np.exp(g64 * (M - Mstar))
        pstar = 8.0 * np.exp(g64[:, None] * (sc_exact - Mstar[:, None]))
        S = S * shift - (pq * shift[:, None]).sum(axis=1) + pstar.sum(axis=1)
        ACC = ACC * shift[:, None] \
            - np.einsum("bk,bkd->bd", pq * shift[:, None],
                        t8f[gidx].astype(np.float64)) \
            + np.einsum("bk,bkd->bd", pstar, tr_top)
        stats.append((Mstar, S, ACC))

    Mg = np.max(np.stack([s[0] for s in stats]), axis=0)
    den = np.zeros(B)
    num = np.zeros((B, D))
    for Mc, S, ACC in stats:
        sl = np.exp(g64 * (Mc - Mg))
        den += sl * S
        num += sl[:, None] * ACC
    weighted = num / den[:, None]

    coef_x = 1.0 / np.sqrt(om)
    coef_x_hat = a / np.sqrt(om)
    out = coef_x[:, None] * xf64 - coef_x_hat[:, None] * weighted
    return out.reshape(x.shape).astype(np.float32)


# revision 18
# speedup vs baseline: 1.6227x; 1.0736x over previous
"""Softmax-weighted nearest-neighbor aggregation (DiffusionStar) on 8 TRN2 cores.

Strategy:
  - Shard the train set (N=50000) across 8 cores (6250 rows each, padded to 6272).
  - All train data streams as fp8 e3m4 (~19.3 MB per copy per core); scores:
      Phase 1: sc[b,n] = <x8_b, t8_n> - (a_b/2)||t_n||^2 via e3m4 GEMM
               (fp32 PSUM) minus a host-prescaled fp32 row; per-group row-max.
      Phase 2: p = 8*exp(gamma*(sc - M)) on ACT (f16 out, ln8 in the bias keeps
               p in (0,8] clear of e3m4 subnormals), p quantized to e3m4 (both
               the transposed GEMM operand and an exported row copy), then
               ACC = p8 @ t8 as an e3m4 GEMM (fp32 PSUM, 49 n-chunks).
  - DRAM layouts are host-pretiled so each dma_start lands as one long
    contiguous run per partition (24 KB for the score stream, 12 KB for the
    natural stream) - descriptor-amortized, near-peak HBM rate.
  - Host merge (fp64): per-core exact top-8 rescore - using the exported p8
    row, the 8 dominant candidates' contributions (weights AND train rows) are
    replaced with exact fp64 values; then the standard online-softmax combine
    across cores. This cancels the fp8 noise exactly where the softmax is
    sharp; the diffuse tail averages it out. Validated ~4e-5 in host sim.
"""

import numpy as np

B = 64
D = 3072
N = 50000
NCORES = 8
N_LOC = N // NCORES          # 6250
N_PAD = 6272                 # 49 * 128
KD = D // 128                # 24
KN = N_PAD // 128            # 49
DJ = D // 512                # 6
NGF = 12                     # full 512-wide groups; last group is 128 wide
GROUPS = [(i * 512, 512) for i in range(NGF)] + [(6144, 128)]
PAD_TRSQ = 1e9
LN_PSCALE = float(np.log(8.0))
TOPK = 8
NAT_Q = 4                    # chunks per natural-stream DMA
NAT_BUFS = 3

_CACHED = {}


def _build_nc():
    import concourse.bacc as bacc
    import concourse.tile as tile
    from concourse import mybir
    from contextlib import ExitStack

    f16 = mybir.dt.float16
    f32 = mybir.dt.float32
    f8 = mybir.dt.float8e3

    nc = bacc.Bacc("TRN2", target_bir_lowering=False, debug=False)

    # host-pretiled DRAM layouts (partition-major, long contiguous runs)
    tTg = nc.dram_tensor("tTg", [128, NGF, KD, 512], f8, kind="ExternalInput").ap()
    tTl = nc.dram_tensor("tTl", [128, KD, 128], f8, kind="ExternalInput").ap()
    natq = nc.dram_tensor("natq", [128, KN, D], f8, kind="ExternalInput").ap()
    xT = nc.dram_tensor("xT", [128, KD, B], f8, kind="ExternalInput").ap()
    ident = nc.dram_tensor("ident", [B, B], f16, kind="ExternalInput").ap()
    trsq = nc.dram_tensor("trsq", [B, N_PAD], f32, kind="ExternalInput").ap()
    gcol = nc.dram_tensor("gcol", [B, 1], f32, kind="ExternalInput").ap()

    acc_out = nc.dram_tensor("acc_out", [B, D], f32, kind="ExternalOutput").ap()
    m_out = nc.dram_tensor("m_out", [B, 1], f32, kind="ExternalOutput").ap()
    p_out = nc.dram_tensor("p_out", [B, N_PAD], f8, kind="ExternalOutput").ap()

    NG = len(GROUPS)

    with tile.TileContext(nc) as tc, ExitStack() as ctx:
        const = ctx.enter_context(tc.tile_pool(name="const", bufs=1))
        kTp = ctx.enter_context(tc.tile_pool(name="kT", bufs=2))
        natp = ctx.enter_context(tc.tile_pool(name="nat", bufs=NAT_BUFS))
        sb = ctx.enter_context(tc.tile_pool(name="sb", bufs=1))
        hip = ctx.enter_context(tc.tile_pool(name="hi", bufs=3))

        # --- constants (trsq is issued after the first kT load below; it is
        #     only needed ~15us in, at the first group's fold) ---
        xT_sb = const.tile([128, KD, B], f8)
        nc.sync.dma_start(xT_sb[:], xT[:])
        id_sb = const.tile([B, B], f16)
        nc.sync.dma_start(id_sb[:], ident[:])
        g_sb = const.tile([B, 1], f32)
        nc.sync.dma_start(g_sb[:], gcol[:])
        trsq_sb = const.tile([B, N_PAD], f32)

        mpart = sb.tile([B, NG], f32)
        stat = sb.tile([B, 4], f32)
        acc_sb = sb.tile([B, D], f32)
        p8row = sb.tile([B, N_PAD], f8)
        sc_tiles = []

        # --- phase 1: scores + per-group max (groups loaded in pairs).
        #     Col-tiled: even k-chunks accumulate on PSUM partitions 0-63,
        #     odd k-chunks on 64-127, concurrently in separate array halves;
        #     DVE folds the two partials while subtracting ||t||^2. ---
        KH = KD // 2
        with tc.tile_pool(name="psS", bufs=3, space="PSUM") as psS:
            for gp in range(NGF // 2):
                kT = kTp.tile([128, 2, KD, 512], f8, tag="kT")
                nc.sync.dma_start(kT[:], tTg[:, 2 * gp:2 * gp + 2])
                if gp == 0:
                    nc.sync.dma_start(trsq_sb[:], trsq[:])
                for gg in range(2):
                    gi = 2 * gp + gg
                    n0 = gi * 512
                    ps = psS.tile([128, 512], f32, tag="ps")
                    for kk in range(KH):
                        nc.tensor.matmul(ps[0:B, :], xT_sb[:, 2 * kk, :],
                                         kT[:, gg, 2 * kk, :],
                                         start=(kk == 0), stop=(kk == KH - 1))
                        nc.tensor.matmul(ps[B:128, :], xT_sb[:, 2 * kk + 1, :],
                                         kT[:, gg, 2 * kk + 1, :],
                                         start=(kk == 0), stop=(kk == KH - 1))
                    sc = sb.tile([B, 512], f32, tag=f"sc{gi}")
                    sc_tiles.append(sc)
                    hi = hip.tile([128, 512], f32, tag="hi")
                    nc.scalar.copy(hi[B:128, :], ps[B:128, :])
                    nc.vector.tensor_tensor(sc[:], ps[0:B, :], hi[B:128, :],
                                            op=mybir.AluOpType.add)
                    nc.vector.tensor_tensor(sc[:], sc[:],
                                            trsq_sb[:, n0:n0 + 512],
                                            op=mybir.AluOpType.subtract)
                    nc.vector.reduce_max(mpart[:, gi:gi + 1], sc[:],
                                         axis=mybir.AxisListType.X)
            # last 128-wide group
            kTe = kTp.tile([128, KD, 128], f8, tag="kTe")
            nc.sync.dma_start(kTe[:], tTl[:])
            ps = psS.tile([128, 512], f32, tag="ps")
            for kk in range(KH):
                nc.tensor.matmul(ps[0:B, :128], xT_sb[:, 2 * kk, :],
                                 kTe[:, 2 * kk, :],
                                 start=(kk == 0), stop=(kk == KH - 1))
                nc.tensor.matmul(ps[B:128, :128], xT_sb[:, 2 * kk + 1, :],
                                 kTe[:, 2 * kk + 1, :],
                                 start=(kk == 0), stop=(kk == KH - 1))
            sc = sb.tile([B, 512], f32, tag=f"sc{NG - 1}")
            sc_tiles.append(sc)
            hi = hip.tile([128, 512], f32, tag="hi")
            nc.scalar.copy(hi[B:128, :128], ps[B:128, :128])
            nc.vector.tensor_tensor(sc[:, :128], ps[0:B, :128], hi[B:128, :128],
                                    op=mybir.AluOpType.add)
            nc.vector.tensor_tensor(sc[:, :128], sc[:, :128],
                                    trsq_sb[:, 6144:6272],
                                    op=mybir.AluOpType.subtract)
            nc.vector.reduce_max(mpart[:, NG - 1:NG], sc[:, :128],
                                 axis=mybir.AxisListType.X)

        # --- global max, bias = -g*M + ln(8) ---
        nc.vector.reduce_max(stat[:, 0:1], mpart[:, :NG],
                             axis=mybir.AxisListType.X)
        nc.vector.tensor_tensor(stat[:, 2:3], g_sb[:], stat[:, 0:1],
                                op=mybir.AluOpType.mult)
        nc.vector.tensor_scalar_mul(stat[:, 2:3], stat[:, 2:3], -1.0)
        nc.vector.tensor_scalar_add(stat[:, 2:3], stat[:, 2:3], LN_PSCALE)

        # --- exp -> e3m4 row copy -> transpose-pair -> col-tiled GEMM2,
        #     interleaved per chunk-pair so the natural stream is consumed
        #     (and its buffers recycled) as early as possible. Even n-chunks
        #     accumulate on PSUM partitions 0-63, odd on 64-127. ---
        with tc.tile_pool(name="psT", bufs=2, space="PSUM") as psT, \
             tc.tile_pool(name="psA", bufs=1, space="PSUM") as psA:
            acc_ps = psA.tile([128, DJ, 512], f32)
            pT_tiles = [None] * KN
            nat_tiles = [None] * KN

            def mm2(c):
                o0 = (c % 2) * B
                for j in range(DJ):
                    nc.tensor.matmul(acc_ps[o0:o0 + B, j, :], pT_tiles[c][:],
                                     nat_tiles[c][:, j * 512:(j + 1) * 512],
                                     start=(c == c % 2), stop=(c >= KN - 2))

            for gi, (n0, W) in enumerate(GROUPS):
                c0 = n0 // 128
                nq = min(NAT_Q, KN - c0)
                natt = natp.tile([128, NAT_Q, D], f8, tag="nat")
                nc.sync.dma_start(natt[:, :nq, :], natq[:, c0:c0 + nq, :])
                for i in range(nq):
                    nat_tiles[c0 + i] = natt[:, i, :]
                p = sb.tile([B, 512], f16, tag=f"p{gi}")
                nc.scalar.activation(p[:, :W], sc_tiles[gi][:, :W],
                                     mybir.ActivationFunctionType.Exp,
                                     bias=stat[:, 2:3], scale=g_sb[:])
                nc.vector.tensor_copy(p8row[:, n0:n0 + W], p[:, :W])
                for ci in range(W // 128):
                    c = c0 + ci
                    pt_ps = psT.tile([128, B], f16, tag="pt")
                    nc.tensor.transpose(pt_ps[:],
                                        p[:, ci * 128:(ci + 1) * 128],
                                        id_sb[:])
                    pT = sb.tile([128, B], f8, tag=f"pT{c}")
                    nc.vector.tensor_copy(pT[:], pt_ps[:])
                    pT_tiles[c] = pT
                    if ci % 2 == 1:          # run the completed pair
                        mm2(c - 1)
                        mm2(c)
                if W % 256 == 128:           # odd trailing chunk (last group)
                    mm2(c0 + W // 128 - 1)
            acc_hi = sb.tile([128, D], f32)
            for j in range(DJ):
                nc.scalar.copy(acc_hi[B:128, j * 512:(j + 1) * 512],
                               acc_ps[B:128, j, :])
                nc.vector.tensor_tensor(acc_sb[:, j * 512:(j + 1) * 512],
                                        acc_ps[0:B, j, :],
                                        acc_hi[B:128, j * 512:(j + 1) * 512],
                                        op=mybir.AluOpType.add)
                nc.sync.dma_start(acc_out[:, j * 512:(j + 1) * 512],
                                  acc_sb[:, j * 512:(j + 1) * 512])

        nc.sync.dma_start(m_out[:], stat[:, 0:1])
        nc.sync.dma_start(p_out[:], p8row[:])

    nc.compile()
    return nc


def _get_nc():
    if "nc" not in _CACHED:
        _CACHED["nc"] = _build_nc()
    return _CACHED["nc"]


def kernel(x, train, alphas_cumprod, t, **_unused):
    import ml_dtypes
    from concourse.bass_utils import run_bass_kernel_spmd

    e3 = ml_dtypes.float8_e3m4

    x = np.asarray(x)
    train = np.asarray(train)
    alphas_cumprod = np.asarray(alphas_cumprod)
    t = np.asarray(t).astype(np.int64)

    xf = x.reshape(B, -1).astype(np.float32)
    tf = train.reshape(N, -1).astype(np.float32)

    acp_t = alphas_cumprod.astype(np.float64)[t]
    a = np.sqrt(acp_t)
    om = 1.0 - acp_t
    g64 = a / om                                     # softmax scale on sc
    gp32 = g64.astype(np.float32)

    trsq_full = np.einsum("nd,nd->n", tf.astype(np.float64),
                          tf.astype(np.float64))

    t8 = tf.astype(e3)
    t8f = t8.astype(np.float32)
    x8 = xf.astype(e3)                               # |x| <= ~4.5, in range
    xTq = np.ascontiguousarray(
        x8.T.reshape(KD, 128, B).transpose(1, 0, 2))  # [128, KD, B]
    ident = np.eye(B, dtype=np.float16)
    g_col = gp32.reshape(B, 1)

    in_maps = []
    for c in range(NCORES):
        sl = slice(c * N_LOC, (c + 1) * N_LOC)
        t8c = np.zeros((N_PAD, D), dtype=e3)
        t8c[:N_LOC] = t8[sl]
        natq_c = np.ascontiguousarray(
            t8c.reshape(KN, 128, D).transpose(1, 0, 2))       # [128, KN, D]
        tTg_c = np.ascontiguousarray(
            t8c[:NGF * 512].reshape(NGF, 512, KD, 128)
            .transpose(3, 0, 2, 1))                  # [128, NGF, KD, 512]
        tTl_c = np.ascontiguousarray(
            t8c[NGF * 512:].reshape(128, KD, 128)
            .transpose(2, 1, 0))                     # [128, KD, 128]
        trsq_c = np.full((N_PAD,), PAD_TRSQ, dtype=np.float64)
        trsq_c[:N_LOC] = trsq_full[sl]
        trsq_c = ((a[:, None] / 2.0) * trsq_c[None, :]).astype(np.float32)
        in_maps.append({
            "tTg": tTg_c,
            "tTl": tTl_c,
            "natq": natq_c,
            "xT": xTq,
            "ident": ident,
            "trsq": np.ascontiguousarray(trsq_c),
            "gcol": g_col,
        })

    nc = _get_nc()
    res = run_bass_kernel_spmd(nc, in_maps, list(range(NCORES)))
    _CACHED["last_results"] = res

    # --- host merge: exact top-K rescore per core + online-softmax combine ---
    xf64 = xf.astype(np.float64)
    stats = []
    for c in range(NCORES):
        M = res.results[c]["m_out"][:, 0].astype(np.float64)
        ACC = res.results[c]["acc_out"].astype(np.float64)
        p8 = np.asarray(res.results[c]["p_out"]).view(e3).astype(np.float32)
        S = p8.astype(np.float64).sum(axis=1)   # consistent with ACC's p8
        idx = np.argpartition(-p8, TOPK, axis=1)[:, :TOPK]
        pq = np.take_along_axis(p8, idx, axis=1).astype(np.float64)
        idx = np.minimum(idx, N_LOC - 1)   # pads only selected when pq == 0
        gidx = idx + c * N_LOC
        tr_top = tf[gidx].astype(np.float64)                  # [B, K, D]
        sc_exact = (np.einsum("bkd,bd->bk", tr_top, xf64)
                    - (a[:, None] / 2.0) * trsq_full[gidx])
        Mstar = np.maximum(M, sc_exact.max(axis=1))
        shift = np.exp(g64 * (M - Mstar))
        pstar = 8.0 * np.exp(g64[:, None] * (sc_exact - Mstar[:, None]))
        S = S * shift - (pq * shift[:, None]).sum(axis=1) + pstar.sum(axis=1)
        ACC = ACC * shift[:, None] \
            - np.einsum("bk,bkd->bd", pq * shift[:, None],
                        t8f[gidx].astype(np.float64)) \
            + np.einsum("bk,bkd->bd", pstar, tr_top)
        stats.append((Mstar, S, ACC))

    Mg = np.max(np.stack([s[0] for s in stats]), axis=0)
    den = np.zeros(B)
    num = np.zeros((B, D))
    for Mc, S, ACC in stats:
        sl = np.exp(g64 * (Mc - Mg))
        den += sl * S
        num += sl[:, None] * ACC
    weighted = num / den[:, None]

    coef_x = 1.0 / np.sqrt(om)
    coef_x_hat = a / np.sqrt(om)
    out = coef_x[:, None] * xf64 - coef_x_hat[:, None] * weighted
    return out.reshape(x.shape).astype(np.float32)


# revision 20
# speedup vs baseline: 1.9170x; 1.1813x over previous
"""Softmax-weighted nearest-neighbor aggregation (DiffusionStar) on 8 TRN2 cores.

Strategy:
  - Shard the train set (N=50000) across 8 cores (6250 rows each, padded to 6272).
  - All train data streams as fp8 e3m4 (~19.3 MB per copy per core); scores:
      Phase 1: sc[b,n] = <x8_b, t8_n> - (a_b/2)||t_n||^2 via e3m4 GEMM
               (fp32 PSUM) minus a host-prescaled fp32 row; per-group row-max.
      Phase 2: p = 8*exp(gamma*(sc - M)) on ACT (f16 out, ln8 in the bias keeps
               p in (0,8] clear of e3m4 subnormals), p quantized to e3m4 (both
               the transposed GEMM operand and an exported row copy), then
               ACC = p8 @ t8 as an e3m4 GEMM (fp32 PSUM, 49 n-chunks).
  - DRAM layouts are host-pretiled so each dma_start lands as one long
    contiguous run per partition (24 KB for the score stream, 12 KB for the
    natural stream) - descriptor-amortized, near-peak HBM rate.
  - Host merge (fp64): per-core exact top-8 rescore - using the exported p8
    row, the 8 dominant candidates' contributions (weights AND train rows) are
    replaced with exact fp64 values; then the standard online-softmax combine
    across cores. This cancels the fp8 noise exactly where the softmax is
    sharp; the diffuse tail averages it out. Validated ~4e-5 in host sim.
"""

import numpy as np

B = 64
D = 3072
N = 50000
NCORES = 8
N_LOC = N // NCORES          # 6250
N_PAD = 6272                 # 49 * 128
KD = D // 128                # 24
KN = N_PAD // 128            # 49
DJ = D // 512                # 6
NGF = 12                     # full 512-wide groups; last group is 128 wide
GROUPS = [(i * 512, 512) for i in range(NGF)] + [(6144, 128)]
PAD_TRSQ = 1e9
LN_PSCALE = float(np.log(8.0))
TOPK = 8
NAT_Q = 4                    # chunks per natural-stream DMA
NAT_BUFS = 4

_CACHED = {}


def _build_nc():
    import concourse.bacc as bacc
    import concourse.tile as tile
    from concourse import mybir
    from contextlib import ExitStack

    f16 = mybir.dt.float16
    f32 = mybir.dt.float32
    f8 = mybir.dt.float8e3

    nc = bacc.Bacc("TRN2", target_bir_lowering=False, debug=False)

    # host-pretiled DRAM layouts (partition-major, long contiguous runs)
    tTg = nc.dram_tensor("tTg", [128, NGF, KD, 512], f8, kind="ExternalInput").ap()
    tTl = nc.dram_tensor("tTl", [128, KD, 128], f8, kind="ExternalInput").ap()
    natq = nc.dram_tensor("natq", [128, KN, D], f8, kind="ExternalInput").ap()
    xT = nc.dram_tensor("xT", [128, KD, B], f8, kind="ExternalInput").ap()
    ident = nc.dram_tensor("ident", [B, B], f16, kind="ExternalInput").ap()
    trsq = nc.dram_tensor("trsq", [B, N_PAD], f16, kind="ExternalInput").ap()
    gcol = nc.dram_tensor("gcol", [B, 1], f32, kind="ExternalInput").ap()

    acc_out = nc.dram_tensor("acc_out", [128, D], f32, kind="ExternalOutput").ap()
    m_out = nc.dram_tensor("m_out", [B, 1], f32, kind="ExternalOutput").ap()
    p_out = nc.dram_tensor("p_out", [B, N_PAD], f8, kind="ExternalOutput").ap()

    NG = len(GROUPS)

    with tile.TileContext(nc) as tc, ExitStack() as ctx:
        const = ctx.enter_context(tc.tile_pool(name="const", bufs=1))
        kTp = ctx.enter_context(tc.tile_pool(name="kT", bufs=2))
        natp = ctx.enter_context(tc.tile_pool(name="nat", bufs=NAT_BUFS))
        sb = ctx.enter_context(tc.tile_pool(name="sb", bufs=1))
        hip = ctx.enter_context(tc.tile_pool(name="hi", bufs=3))

        # --- constants (trsq is issued after the first kT load below; it is
        #     only needed ~15us in, at the first group's fold) ---
        kT0 = kTp.tile([128, 2, KD, 512], f8, tag="kT")
        nc.sync.dma_start(kT0[:], tTg[:, 0:2])
        xT_sb = const.tile([128, KD, B], f8)
        nc.scalar.dma_start(xT_sb[:], xT[:])
        id_sb = const.tile([B, B], f16)
        nc.scalar.dma_start(id_sb[:], ident[:])
        g_sb = const.tile([B, 1], f32)
        nc.scalar.dma_start(g_sb[:], gcol[:])
        trsq_sb = const.tile([B, N_PAD], f16)

        mpart = sb.tile([B, NG], f32)
        stat = sb.tile([B, 4], f32)
        acc_sb = sb.tile([128, D], f32)
        p8row = sb.tile([B, N_PAD], f8)
        sc_tiles = []

        # --- phase 1: scores + per-group max (groups loaded in pairs).
        #     Col-tiled: even k-chunks accumulate on PSUM partitions 0-63,
        #     odd k-chunks on 64-127, concurrently in separate array halves;
        #     DVE folds the two partials while subtracting ||t||^2. ---
        KH = KD // 2
        with tc.tile_pool(name="psS", bufs=3, space="PSUM") as psS:
            for gp in range(NGF // 2):
                if gp == 0:
                    kT = kT0
                    nc.scalar.dma_start(trsq_sb[:], trsq[:])
                else:
                    kT = kTp.tile([128, 2, KD, 512], f8, tag="kT")
                    nc.sync.dma_start(kT[:], tTg[:, 2 * gp:2 * gp + 2])
                for gg in range(2):
                    gi = 2 * gp + gg
                    n0 = gi * 512
                    ps = psS.tile([128, 512], f32, tag="ps")
                    for kk in range(KH):
                        nc.tensor.matmul(ps[0:B, :], xT_sb[:, 2 * kk, :],
                                         kT[:, gg, 2 * kk, :],
                                         start=(kk == 0), stop=(kk == KH - 1))
                        nc.tensor.matmul(ps[B:128, :], xT_sb[:, 2 * kk + 1, :],
                                         kT[:, gg, 2 * kk + 1, :],
                                         start=(kk == 0), stop=(kk == KH - 1))
                    sc = sb.tile([B, 512], f32, tag=f"sc{gi}")
                    sc_tiles.append(sc)
                    hi = hip.tile([128, 512], f32, tag="hi")
                    nc.scalar.copy(hi[B:128, :], ps[B:128, :])
                    nc.vector.tensor_tensor(sc[:], ps[0:B, :], hi[B:128, :],
                                            op=mybir.AluOpType.add)
                    nc.vector.tensor_tensor(sc[:], sc[:],
                                            trsq_sb[:, n0:n0 + 512],
                                            op=mybir.AluOpType.subtract)
                    nc.vector.reduce_max(mpart[:, gi:gi + 1], sc[:],
                                         axis=mybir.AxisListType.X)
            # last 128-wide group
            kTe = kTp.tile([128, KD, 128], f8, tag="kTe")
            nc.sync.dma_start(kTe[:], tTl[:])
            ps = psS.tile([128, 512], f32, tag="ps")
            for kk in range(KH):
                nc.tensor.matmul(ps[0:B, :128], xT_sb[:, 2 * kk, :],
                                 kTe[:, 2 * kk, :],
                                 start=(kk == 0), stop=(kk == KH - 1))
                nc.tensor.matmul(ps[B:128, :128], xT_sb[:, 2 * kk + 1, :],
                                 kTe[:, 2 * kk + 1, :],
                                 start=(kk == 0), stop=(kk == KH - 1))
            sc = sb.tile([B, 512], f32, tag=f"sc{NG - 1}")
            sc_tiles.append(sc)
            hi = hip.tile([128, 512], f32, tag="hi")
            nc.scalar.copy(hi[B:128, :128], ps[B:128, :128])
            nc.vector.tensor_tensor(sc[:, :128], ps[0:B, :128], hi[B:128, :128],
                                    op=mybir.AluOpType.add)
            nc.vector.tensor_tensor(sc[:, :128], sc[:, :128],
                                    trsq_sb[:, 6144:6272],
                                    op=mybir.AluOpType.subtract)
            nc.vector.reduce_max(mpart[:, NG - 1:NG], sc[:, :128],
                                 axis=mybir.AxisListType.X)

        # --- global max, bias = -g*M + ln(8) ---
        nc.vector.reduce_max(stat[:, 0:1], mpart[:, :NG],
                             axis=mybir.AxisListType.X)
        nc.vector.tensor_tensor(stat[:, 2:3], g_sb[:], stat[:, 0:1],
                                op=mybir.AluOpType.mult)
        nc.vector.tensor_scalar_mul(stat[:, 2:3], stat[:, 2:3], -1.0)
        nc.vector.tensor_scalar_add(stat[:, 2:3], stat[:, 2:3], LN_PSCALE)

        # --- exp -> e3m4 row copy -> transpose-pair -> col-tiled GEMM2,
        #     interleaved per chunk-pair so the natural stream is consumed
        #     (and its buffers recycled) as early as possible. Even n-chunks
        #     accumulate on PSUM partitions 0-63, odd on 64-127. ---
        with tc.tile_pool(name="psT", bufs=2, space="PSUM") as psT, \
             tc.tile_pool(name="psA", bufs=1, space="PSUM") as psA:
            acc_ps = psA.tile([128, DJ, 512], f32)
            pT_tiles = [None] * KN
            nat_tiles = [None] * KN

            def mm2(c):
                o0 = (c % 2) * B
                for j in range(DJ):
                    nc.tensor.matmul(acc_ps[o0:o0 + B, j, :], pT_tiles[c][:],
                                     nat_tiles[c][:, j * 512:(j + 1) * 512],
                                     start=(c == c % 2), stop=(c >= KN - 2))

            for gi, (n0, W) in enumerate(GROUPS):
                c0 = n0 // 128
                nq = min(NAT_Q, KN - c0)
                natt = natp.tile([128, NAT_Q, D], f8, tag="nat")
                nc.sync.dma_start(natt[:, :nq, :], natq[:, c0:c0 + nq, :])
                for i in range(nq):
                    nat_tiles[c0 + i] = natt[:, i, :]
                p = sb.tile([B, 512], f16, tag=f"p{gi}")
                nc.scalar.activation(p[:, :W], sc_tiles[gi][:, :W],
                                     mybir.ActivationFunctionType.Exp,
                                     bias=stat[:, 2:3], scale=g_sb[:])
                nc.vector.tensor_copy(p8row[:, n0:n0 + W], p[:, :W])
                ncH = W // 128
                pt_ps = psT.tile([128, 4, B], f16, tag="pt")
                for ci in range(ncH):
                    nc.tensor.transpose(pt_ps[:, ci, :],
                                        p[:, ci * 128:(ci + 1) * 128],
                                        id_sb[:])
                pT4 = sb.tile([128, 4, B], f8, tag=f"pT{gi}")
                nc.vector.tensor_copy(pT4[:, :ncH, :], pt_ps[:, :ncH, :])
                for ci in range(ncH):
                    pT_tiles[c0 + ci] = pT4[:, ci, :]
                for pc in range(ncH // 2):
                    mm2(c0 + 2 * pc)
                    mm2(c0 + 2 * pc + 1)
                if ncH % 2:                  # odd trailing chunk (last group)
                    mm2(c0 + ncH - 1)
            for j in range(DJ):
                nc.scalar.copy(acc_sb[:, j * 512:(j + 1) * 512],
                               acc_ps[:, j, :])
                nc.sync.dma_start(acc_out[:, j * 512:(j + 1) * 512],
                                  acc_sb[:, j * 512:(j + 1) * 512])

        nc.scalar.dma_start(m_out[:], stat[:, 0:1])
        nc.scalar.dma_start(p_out[:], p8row[:])

    nc.compile()
    return nc


def _get_nc():
    if "nc" not in _CACHED:
        _CACHED["nc"] = _build_nc()
    return _CACHED["nc"]


def kernel(x, train, alphas_cumprod, t, **_unused):
    import ml_dtypes
    from concourse.bass_utils import run_bass_kernel_spmd

    e3 = ml_dtypes.float8_e3m4

    x = np.asarray(x)
    train = np.asarray(train)
    alphas_cumprod = np.asarray(alphas_cumprod)
    t = np.asarray(t).astype(np.int64)

    xf = x.reshape(B, -1).astype(np.float32)
    tf = train.reshape(N, -1).astype(np.float32)

    acp_t = alphas_cumprod.astype(np.float64)[t]
    a = np.sqrt(acp_t)
    om = 1.0 - acp_t
    g64 = a / om                                     # softmax scale on sc
    gp32 = g64.astype(np.float32)

    trsq_full = np.einsum("nd,nd->n", tf.astype(np.float64),
                          tf.astype(np.float64))

    t8 = tf.astype(e3)
    t8f = t8.astype(np.float32)
    x8 = xf.astype(e3)                               # |x| <= ~4.5, in range
    xTq = np.ascontiguousarray(
        x8.T.reshape(KD, 128, B).transpose(1, 0, 2))  # [128, KD, B]
    ident = np.eye(B, dtype=np.float16)
    g_col = gp32.reshape(B, 1)

    in_maps = []
    for c in range(NCORES):
        sl = slice(c * N_LOC, (c + 1) * N_LOC)
        t8c = np.zeros((N_PAD, D), dtype=e3)
        t8c[:N_LOC] = t8[sl]
        natq_c = np.ascontiguousarray(
            t8c.reshape(KN, 128, D).transpose(1, 0, 2))       # [128, KN, D]
        tTg_c = np.ascontiguousarray(
            t8c[:NGF * 512].reshape(NGF, 512, KD, 128)
            .transpose(3, 0, 2, 1))                  # [128, NGF, KD, 512]
        tTl_c = np.ascontiguousarray(
            t8c[NGF * 512:].reshape(128, KD, 128)
            .transpose(2, 1, 0))                     # [128, KD, 128]
        trsq_c = np.full((N_PAD,), PAD_TRSQ, dtype=np.float64)
        trsq_c[:N_LOC] = (a.min() / 2.0) * trsq_full[sl]
        trsq_c = np.broadcast_to(trsq_c[None, :], (B, N_PAD)).copy()
        trsq_c[:, :N_LOC] = (a[:, None] / 2.0) * trsq_full[None, sl]
        trsq_c = np.minimum(trsq_c, 60000.0).astype(np.float16)
        in_maps.append({
            "tTg": tTg_c,
            "tTl": tTl_c,
            "natq": natq_c,
            "xT": xTq,
            "ident": ident,
            "trsq": np.ascontiguousarray(trsq_c),
            "gcol": g_col,
        })

    nc = _get_nc()
    res = run_bass_kernel_spmd(nc, in_maps, list(range(NCORES)))
    _CACHED["last_results"] = res

    # --- host merge: exact top-K rescore per core + online-softmax combine ---
    xf64 = xf.astype(np.float64)
    stats = []
    for c in range(NCORES):
        M = res.results[c]["m_out"][:, 0].astype(np.float64)
        acc2 = res.results[c]["acc_out"].astype(np.float64)
        ACC = acc2[0:B] + acc2[B:128]
        p8 = np.asarray(res.results[c]["p_out"]).view(e3).astype(np.float32)
        S = p8.astype(np.float64).sum(axis=1)   # consistent with ACC's p8
        idx = np.argpartition(-p8, TOPK, axis=1)[:, :TOPK]
        pq = np.take_along_axis(p8, idx, axis=1).astype(np.float64)
        idx = np.minimum(idx, N_LOC - 1)   # pads only selected when pq == 0
        gidx = idx + c * N_LOC
        tr_top = tf[gidx].astype(np.float64)                  # [B, K, D]
        sc_exact = (np.einsum("bkd,bd->bk", tr_top, xf64)
                    - (a[:, None] / 2.0) * trsq_full[gidx])
        Mstar = np.maximum(M, sc_exact.max(axis=1))
        shift = np.exp(g64 * (M - Mstar))
        pstar = 8.0 * np.exp(g64[:, None] * (sc_exact - Mstar[:, None]))
        S = S * shift - (pq * shift[:, None]).sum(axis=1) + pstar.sum(axis=1)
        ACC = ACC * shift[:, None] \
            - np.einsum("bk,bkd->bd", pq * shift[:, None],
                        t8f[gidx].astype(np.float64)) \
            + np.einsum("bk,bkd->bd", pstar, tr_top)
        stats.append((Mstar, S, ACC))

    Mg = np.max(np.stack([s[0] for s in stats]), axis=0)
    den = np.zeros(B)
    num = np.zeros((B, D))
    for Mc, S, ACC in stats:
        sl = np.exp(g64 * (Mc - Mg))
        den += sl * S
        num += sl[:, None] * ACC
    weighted = num / den[:, None]

    coef_x = 1.0 / np.sqrt(om)
    coef_x_hat = a / np.sqrt(om)
    out = coef_x[:, None] * xf64 - coef_x_hat[:, None] * weighted
    return out.reshape(x.shape).astype(np.float32)
